# revision 21
# baseline (speedup 1.0000x reference)
"""Trainium2 Bass kernel for nn_ABMIL_UCB (topk_masking).

8-core SPMD, data-parallel over N (patches). Per core:
  phase 1: stream the 32MB feature shard once; PE-transpose tiles; fp32r
           matmuls X@W1^T -> tanh(+b1) -> Wb-dot => fast S [16bh, 4096];
           featsum accumulated by the ACT transpose-copies (accum_out).
  phase 2: local top-16 candidates per (b,h) via DVE max8/max_index;
           exact-fp32 rescue recompute of candidate scores (dma_gather rows,
           fp32 matmuls + ACT tanh); ONE AllGather of
           (exact candidate S, global idx, featsum partials).
  phase 3 (replicated on all cores): global exact top-10 per (b,h) using the
           reference's ucb (+const per (b,h)) fp32 bucketing and lowest-index
           tie-breaks; masked-softmax pooling via the algebraic form
           weighted = (featsum + sum_j (e^{s~_j}-1) x_j) / (N-K+sum_j e^{s~_j});
           agg = Wc @ weighted (fo-sharded across cores); one-hot count update.
Host only shards/reshapes inputs and reassembles outputs.
"""

import sys
import os

for _p in ("/opt/trn_rl_repo", "/root/.axon_site/_ro/trn_rl_repo"):
    if os.path.isdir(_p) and _p not in sys.path:
        sys.path.append(_p)

import numpy as np

from concourse import bass, mybir, masks
from concourse import tile
from concourse.bass_utils import run_bass_kernel_spmd

DT = mybir.dt.float32
FR = mybir.dt.float32r
I16 = mybir.dt.int16
I32 = mybir.dt.int32
U32 = mybir.dt.uint32
BF = mybir.dt.bfloat16
AF = mybir.ActivationFunctionType
ALU = mybir.AluOpType
AX = mybir.AxisListType

B, N, F, HD, H = 2, 32768, 1024, 256, 8
NCORES = 8
NL = N // NCORES            # 4096 patches per core
COLS = H * HD               # 2048
NBLK = 16                   # row blocks of 512 (8 per b)
RB = 512
TOPK = 10
CAND = 16                   # local candidates per (b, h)
NEG = -1.0e30
INF = 3.0e38

_CACHE = {}


def _split_multi_waits(nc, max_waits=1):
    """This neuronxcc build accepts only one sync-wait per instruction;
    move extra waits onto preceding wait-only NoOps."""
    n = 0
    for bb in nc.main_func.blocks:
        insts = bb.instructions
        new = []
        changed = False
        for ins in insts:
            si = getattr(ins, "sync_info", None)
            if si is not None and si.on_wait and len(si.on_wait) > max_waits:
                waits = list(si.on_wait)
                for w in waits[:-max_waits]:
                    nop = mybir.InstNoOp(
                        name=f"I-waitsplit-{nc.next_id()}",
                        sync_info=mybir.SyncInfo(on_wait=[w], on_update=[]),
                        bass_nofuse=True,
                        engine=ins.engine,
                    )
                    nc.register_instruction(nop, overwrite=True)
                    new.append(nop)
                    n += 1
                ins.sync_info = mybir.SyncInfo(
                    on_wait=waits[-max_waits:],
                    on_update=list(si.on_update) if si.on_update else [],
                )
                changed = True
            new.append(ins)
        if changed:
            bb.instructions = new
    return n


def _build():
    core_ids = list(range(NCORES))
    nc = bass.Bass()

    feat_in = nc.declare_dram_parameter("feat", [B, NL, F], DT, isOutput=False)
    w1_in = nc.declare_dram_parameter("W1m", [8, 128, 8, 256], DT, isOutput=False)
    w1b_in = nc.declare_dram_parameter("W1mb", [8, 128, COLS], BF, isOutput=False)
    b1r_in = nc.declare_dram_parameter("b1rep", [16, COLS], DT, isOutput=False)
    wbr_in = nc.declare_dram_parameter("Wbrep", [16, COLS], DT, isOutput=False)
    bcr_in = nc.declare_dram_parameter("bcrep", [2, 128], DT, isOutput=False)
    b1_in = nc.declare_dram_parameter("b1T", [128, 16], DT, isOutput=False)
    wbg_in = nc.declare_dram_parameter("Wbg", [16, 128, 8], DT, isOutput=False)
    wbm_in = nc.declare_dram_parameter("Wbm", [128, 2, 8], DT, isOutput=False)
    bbv_in = nc.declare_dram_parameter("bbv", [8, 1], DT, isOutput=False)
    bb16_in = nc.declare_dram_parameter("bb16", [16, 1], DT, isOutput=False)
    wct_in = nc.declare_dram_parameter("WcT", [8, 128, 1024], DT, isOutput=False)
    bcs_in = nc.declare_dram_parameter("bcs", [128, 1], DT, isOutput=False)
    cnt_in = nc.declare_dram_parameter("cnt", [B, NL, H], DT, isOutput=False)
    cvec_in = nc.declare_dram_parameter("cvec", [16, 1], DT, isOutput=False)
    # meta[:,0]=rowbase (core*NL + (p%8)*512); meta[:,1]=coreoff (core*NL);
    # meta[:,2]= (p<16): (p//8)*NL else 0; meta[:,3]=coreoff+NL
    meta_in = nc.declare_dram_parameter("meta", [128, 4], DT, isOutput=False)
    eqpa_in = nc.declare_dram_parameter("eqpA", [128, 16], DT, isOutput=False)
    eqpb_in = nc.declare_dram_parameter("eqpB", [128, 16], DT, isOutput=False)

    s_out = nc.declare_dram_parameter("S_out", [16, NL], DT, isOutput=True)
    cnt_out = nc.declare_dram_parameter("cnt_out", [B, NL, H], DT, isOutput=True)
    agg_out = nc.declare_dram_parameter("agg_out", [2, 128], DT, isOutput=True)

    with tile.TileContext(nc) as tc:
        with tc.tile_pool(name="const", bufs=1) as cpool, \
             tc.tile_pool(name="work", bufs=2) as wpool, \
             tc.tile_pool(name="nat", bufs=2) as npool, \
             tc.tile_pool(name="avp", bufs=3) as apool, \
             tc.tile_pool(name="small", bufs=1) as spool, \
             tc.tile_pool(name="ps", bufs=3, space="PSUM") as ps, \
             tc.tile_pool(name="psS", bufs=1, space="PSUM") as psS, \
             tc.tile_pool(name="dram", bufs=1, space="DRAM") as dpool:

            ident = cpool.tile([128, 128], DT, tag="ident")
            masks.make_identity(nc, ident[:])

            identB = cpool.tile([128, 128], BF, tag="identB")
            masks.make_identity(nc, identB[:])
            w1 = cpool.tile([128, 8, COLS], BF, tag="w1")
            for k in range(8):
                nc.sync.dma_start(w1[:, k], w1b_in[k])
            b1rep = cpool.tile([16, COLS], DT, tag="b1rep")
            nc.sync.dma_start(b1rep[:], b1r_in[:])
            wbrep = cpool.tile([16, COLS], DT, tag="wbrep")
            nc.sync.dma_start(wbrep[:], wbr_in[:])
            bcrep = cpool.tile([2, 128], DT, tag="bcrep")
            nc.sync.dma_start(bcrep[:], bcr_in[:])
            wbig = cpool.tile([128, 16, 8], FR, tag="wbig")
            for m in range(16):
                nc.gpsimd.dma_start(wbig[:, m], wbg_in[m])
            b1T = cpool.tile([128, 16], DT, tag="b1T")
            nc.sync.dma_start(b1T[:], b1_in[:])
            wbm = cpool.tile([128, 2, 8], DT, tag="wbm")
            nc.sync.dma_start(wbm[:], wbm_in[:])
            bbv = cpool.tile([8, 1], DT, tag="bbv")
            nc.sync.dma_start(bbv[:], bbv_in[:])
            bb16 = cpool.tile([16, 1], DT, tag="bb16")
            nc.sync.dma_start(bb16[:], bb16_in[:])
            bcs = cpool.tile([128, 1], DT, tag="bcs")
            nc.sync.dma_start(bcs[:], bcs_in[:])
            cvec = cpool.tile([16, 1], DT, tag="cvec")
            nc.sync.dma_start(cvec[:], cvec_in[:])
            meta = cpool.tile([128, 4], DT, tag="meta")
            nc.sync.dma_start(meta[:], meta_in[:])
            eqpA = cpool.tile([128, 16], DT, tag="eqpA")
            nc.sync.dma_start(eqpA[:], eqpa_in[:])
            eqpB = cpool.tile([128, 16], DT, tag="eqpB")
            nc.sync.dma_start(eqpB[:], eqpb_in[:])
            onesr = cpool.tile([1, 128], DT, tag="onesr")
            nc.vector.memset(onesr[:], 1.0)

            accfs = spool.tile([128, NBLK, 8, 4], DT, tag="accfs")
            sdr = dpool.tile([16, NL], DT)

            # ---------------- phase 1 ----------------
            # per-block transpose emitters so transposes interleave with the
            # previous block's matmuls (keeps the PE HAM clock warm)
            def make_xtr_ops(r):
                b, rb = r // 8, r % 8
                Xtr = wpool.tile([128, 8, RB], BF, tag="Xtr", name=f"Xtr{r}")
                ops = []
                for t in range(4):
                    natX = npool.tile([128, F], DT, tag="natX", name=f"natX{r}_{t}")
                    nc.sync.dma_start(
                        natX[:], feat_in[b, rb * RB + t * 128: rb * RB + (t + 1) * 128, :])
                    natB = npool.tile([128, F], BF, tag="natB", name=f"natB{r}_{t}")
                    nc.vector.tensor_copy(natB[:], natX[:])

                    def tr_op(t=t, k=None, natB=natB, Xtr=Xtr, r=r):
                        for k in range(8):
                            ptr = ps.tile([128, 128], BF, tag="ptr", bufs=2,
                                          name=f"ptr{r}_{t}_{k}")
                            nc.tensor.transpose(ptr[:], natB[:, k * 128:(k + 1) * 128], identB[:])
                            nc.scalar.activation(
                                Xtr[:, k, t * 128:(t + 1) * 128], ptr[:], AF.Copy,
                                accum_out=accfs[:, r, k, t:t + 1])
                    ops.append(tr_op)
                return Xtr, ops

            pend = None  # delayed S-matmul: (m, avt, pS)
            Xtr_cur, ops0 = make_xtr_ops(0)
            for op in ops0:
                op()
            for r in range(NBLK):
                b, rb = r // 8, r % 8
                if r + 1 < NBLK:
                    Xtr_next, next_ops = make_xtr_ops(r + 1)
                else:
                    Xtr_next, next_ops = None, []
                pS = psS.tile([8, RB], DT, tag=f"pS{r % 2}", name=f"pS{r}")
                for m in range(16):
                    pt = ps.tile([128, RB], DT, tag="pt", name=f"pt{r}_{m}")
                    for k in range(8):
                        nc.tensor.matmul(pt[:], lhsT=w1[:, k, m * 128:(m + 1) * 128],
                                         rhs=Xtr_cur[:, k], start=(k == 0), stop=(k == 7))
                    if pend is not None:
                        pm, pavt, ppS = pend
                        nc.tensor.matmul(ppS[:], lhsT=wbig[:, pm], rhs=pavt[:],
                                         start=(pm == 0), stop=(pm == 15))
                    # sprinkle next block's transposes between m-tiles
                    if m % 4 == 1 and next_ops:
                        next_ops.pop(0)()
                    avt = apool.tile([128, RB], FR, tag="avt", name=f"avt{r}_{m}")
                    nc.scalar.activation(avt[:], pt[:], AF.Tanh, bias=b1T[:, m:m + 1])
                    pend = (m, avt, pS)
                pm, pavt, ppS = pend
                nc.tensor.matmul(ppS[:], lhsT=wbig[:, pm], rhs=pavt[:],
                                 start=(pm == 0), stop=(pm == 15))
                pend = None
                for op in next_ops:
                    op()
                sstage = apool.tile([8, RB], DT, tag="sstage", name=f"sstage{r}")
                nc.vector.tensor_scalar(sstage[:], pS[:], bbv[:, 0:1], None, op0=ALU.add)
                nc.sync.dma_start(sdr[b * 8:(b + 1) * 8, rb * RB:(rb + 1) * RB], sstage[:])
                Xtr_cur = Xtr_next

            # featsum partials: fs[p, k, b]
            fs = spool.tile([128, 8, 2], DT, tag="fs")
            nc.vector.tensor_reduce(
                fs[:], accfs[:].rearrange("p (b blk) k t -> p k b blk t", b=2),
                axis=AX.XY, op=ALU.add)

            # ---------------- phase 2: local candidates + rescue ----------------
            nc.sync.dma_start(s_out[:], sdr[:])
            wk = spool.tile([16, NL], DT, tag="wkt")
            nc.sync.dma_start(wk[:], sdr[:])
            mx = spool.tile([16, CAND], DT, tag="mx")
            ixu = spool.tile([16, CAND], U32, tag="ixu")
            for r0 in range(CAND // 8):
                nc.vector.max(mx[:, r0 * 8:(r0 + 1) * 8], wk[:])
                nc.vector.max_index(ixu[:, r0 * 8:(r0 + 1) * 8], mx[:, r0 * 8:(r0 + 1) * 8], wk[:])
                if r0 + 1 < CAND // 8:
                    nc.vector.match_replace(wk[:], mx[:, r0 * 8:(r0 + 1) * 8], wk[:], NEG)

            ixf = spool.tile([16, CAND], DT, tag="ixf")
            nc.vector.tensor_copy(ixf[:], ixu[:])
            gidx = spool.tile([16, CAND], DT, tag="gidx")
            nc.vector.tensor_scalar(gidx[:], ixf[:], meta[0:16, 1:2], None, op0=ALU.add)
            # row index into flattened [B*NL, F]: + b*NL
            ixrow = spool.tile([16, CAND], DT, tag="ixrow")
            nc.vector.tensor_scalar(ixrow[:], ixf[:], meta[0:16, 2:3], None, op0=ALU.add)
            # gather slot g = b*128 + (h*16+i): flat DRAM roundtrip gives per-b
            # [128,1] per-partition index tiles
            ixdr = dpool.tile([1, 256], DT)
            nc.sync.dma_start(ixdr[0, :].rearrange("(bh i) -> bh i", bh=16), ixrow[:])
            gx = spool.tile([128, 2, F], DT, tag="gx")
            for b2 in range(2):
                ixl = spool.tile([128, 1], DT, tag=f"ixl{b2}")
                nc.sync.dma_start(ixl[:], ixdr[0, b2 * 128:(b2 + 1) * 128].rearrange("(p one) -> p one", one=1))
                ixi = spool.tile([128, 1], I32, tag=f"ixi{b2}")
                nc.vector.tensor_copy(ixi[:], ixl[:])
                nc.gpsimd.indirect_dma_start(
                    out=gx[:, b2, :], out_offset=None,
                    in_=feat_in[:].rearrange("b n f -> (b n) f"),
                    in_offset=bass.IndirectOffsetOnAxis(ap=ixi[:, 0:1], axis=0))
            # transpose gathered rows -> XcT[p=f%128, (c2b, k), 128 rows]
            XcT = spool.tile([128, 2, 8, 128], DT, tag="XcT")
            for c2 in range(2):
                for k in range(8):
                    ptr = ps.tile([128, 128], DT, tag="ptr", bufs=2)
                    nc.tensor.transpose(ptr[:], gx[:, c2, k * 128:(k + 1) * 128], ident[:])
                    nc.scalar.activation(XcT[:, c2, k], ptr[:], AF.Copy)
            # exact per-head S for candidates: av^T-cand[16 rows, 256 d] via
            # stationary-XcT matmuls (tiny LDWEIGHTS), Wb-dot on DVE
            sxdr = dpool.tile([16, CAND], DT)
            for h in range(8):
                w1h = wpool.tile([128, 8, 256], DT, tag="w1h", bufs=2)
                nc.sync.dma_start(w1h[:], w1_in[h])
                for b2 in range(2):
                    pa = ps.tile([16, 256], DT, tag="pa", bufs=1)
                    for k in range(8):
                        nc.tensor.matmul(
                            pa[:], lhsT=XcT[:, b2, k, h * 16:(h + 1) * 16],
                            rhs=w1h[:, k], start=(k == 0), stop=(k == 7))
                    avc = apool.tile([16, 256], DT, tag="avc")
                    nc.vector.tensor_tensor(out=avc[:], in0=pa[:],
                                            in1=b1rep[:, h * 256:(h + 1) * 256], op=ALU.add)
                    nc.scalar.activation(avc[:], avc[:], AF.Tanh)
                    nc.vector.tensor_tensor(out=avc[:], in0=avc[:],
                                            in1=wbrep[:, h * 256:(h + 1) * 256], op=ALU.mult)
                    scand = apool.tile([16, 1], DT, tag="scand")
                    nc.vector.tensor_reduce(scand[:], avc[:], axis=AX.X, op=ALU.add)
                    nc.sync.dma_start(
                        sxdr[b2 * 8 + h, :].rearrange("(i one) -> i one", one=1), scand[:])
            Sx = spool.tile([16, CAND], DT, tag="Sx")
            nc.sync.dma_start(Sx[:], sxdr[:])
            nc.vector.tensor_scalar(Sx[:], Sx[:], bb16[:, 0:1], None, op0=ALU.add)

            # ---------------- AllGather ----------------
            PAY = 16 * CAND * 2 + 2048       # 2560 fp32
            agin = dpool.tile([1, PAY], DT)
            agout = dpool.tile([NCORES, PAY], DT)
            nc.sync.dma_start(agin[0, 0:256].rearrange("(bh i) -> bh i", bh=16), Sx[:])
            nc.sync.dma_start(agin[0, 256:512].rearrange("(bh i) -> bh i", bh=16), gidx[:])
            for b in range(2):
                nc.sync.dma_start(
                    agin[0, 512 + b * 1024: 512 + (b + 1) * 1024].rearrange("(k p) -> p k", k=8),
                    fs[:, :, b])
            nc.gpsimd.collective_compute(
                "AllGather", ALU.bypass, replica_groups=[core_ids],
                ins=[agin.opt()], outs=[agout.opt()])

            NCAND = NCORES * CAND  # 128
            SallC = spool.tile([16, NCAND], DT, tag="SallC")
            nc.sync.dma_start(
                SallC[:].rearrange("bh (c i) -> bh c i", c=NCORES),
                agout[:, 0:256].rearrange("c (bh i) -> bh c i", bh=16))
            Gall = spool.tile([16, NCAND], DT, tag="Gall")
            nc.sync.dma_start(
                Gall[:].rearrange("bh (c i) -> bh c i", c=NCORES),
                agout[:, 256:512].rearrange("c (bh i) -> bh c i", bh=16))
            fsnat = spool.tile([8, 2048], DT, tag="fsnat")
            nc.sync.dma_start(fsnat[:], agout[:, 512:2560])
            ones8 = spool.tile([8, 1], DT, tag="ones8")
            nc.vector.memset(ones8[:], 1.0)
            pfs = psS.tile([128, 16], DT, tag="pS0")
            for kb in range(16):
                nc.tensor.matmul(pfs[:, kb:kb + 1], lhsT=fsnat[:, kb * 128:(kb + 1) * 128],
                                 rhs=ones8[:], start=True, stop=True)
            # pfs[p, kb] with kb = b*8+k ordering from payload (b k p)
            fsG = spool.tile([128, 8, 2], DT, tag="fsG")
            nc.vector.tensor_copy(fsG[:].rearrange("p k b -> p b k"),
                                  pfs[:].rearrange("p (b k) -> p b k", b=2))

            # ---------------- phase 3: global select (replicated) ----------------
            ucb = spool.tile([16, NCAND], DT, tag="ucb")
            nc.vector.tensor_scalar(ucb[:], SallC[:], cvec[:, 0:1], None, op0=ALU.add)
            inft = spool.tile([16, NCAND], DT, tag="inft")
            nc.vector.memset(inft[:], INF)
            ninft = spool.tile([16, NCAND], DT, tag="ninft")
            nc.vector.memset(ninft[:], NEG)
            selv = spool.tile([16, TOPK], DT, tag="selv")
            selg = spool.tile([16, TOPK], DT, tag="selg")
            mxv = spool.tile([16, 1], DT, tag="mxv")
            gmin = spool.tile([16, 1], DT, tag="gmin")
            eqv = spool.tile([16, NCAND], mybir.dt.uint8, tag="eqv")
            tmpm = spool.tile([16, NCAND], DT, tag="tmpm")
            for j in range(TOPK):
                nc.vector.tensor_reduce(mxv[:], ucb[:], axis=AX.X, op=ALU.max)
                nc.vector.tensor_scalar(eqv[:], ucb[:], mxv[:, 0:1], None, op0=ALU.is_ge)
                nc.vector.select(tmpm[:], eqv[:], Gall[:], inft[:])
                nc.vector.tensor_reduce(gmin[:], tmpm[:], axis=AX.X, op=ALU.min)
                nc.vector.tensor_copy(selg[:, j:j + 1], gmin[:])
                nc.vector.tensor_scalar(eqv[:], Gall[:], gmin[:, 0:1], None, op0=ALU.is_equal)
                nc.vector.select(tmpm[:], eqv[:], SallC[:], ninft[:])
                nc.vector.tensor_reduce(selv[:, j:j + 1], tmpm[:], axis=AX.X, op=ALU.max)
                nc.vector.copy_predicated(ucb[:], eqv[:], ninft[:])

            # pooling scalars
            ssum = spool.tile([16, 1], DT, tag="ssum")
            nc.vector.tensor_reduce(ssum[:], selv[:], axis=AX.X, op=ALU.add)
            nc.vector.tensor_scalar(ssum[:], ssum[:], 1e-6, None, op0=ALU.add)
            rs = spool.tile([16, 1], DT, tag="rs")
            nc.vector.reciprocal(rs[:], ssum[:])
            stl = spool.tile([16, TOPK], DT, tag="stl")
            nc.vector.tensor_scalar(stl[:], selv[:], rs[:, 0:1], None, op0=ALU.mult)
            exv = spool.tile([16, TOPK], DT, tag="exv")
            nc.scalar.activation(exv[:], stl[:], AF.Exp)
            zs = spool.tile([16, 1], DT, tag="zs")
            nc.vector.tensor_reduce(zs[:], exv[:], axis=AX.X, op=ALU.add)
            nc.vector.tensor_scalar(zs[:], zs[:], float(N - TOPK), None, op0=ALU.add)
            zinv = spool.tile([16, 1], DT, tag="zinv")
            nc.vector.reciprocal(zinv[:], zs[:])
            alpha = spool.tile([16, TOPK], DT, tag="alpha")
            nc.vector.tensor_scalar(alpha[:], exv[:], -1.0, None, op0=ALU.add)
            nc.vector.tensor_scalar(alpha[:], alpha[:], zinv[:, 0:1], None, op0=ALU.mult)
            # ownership mask and local row indices
            own = spool.tile([16, TOPK], DT, tag="own")
            t2 = spool.tile([16, TOPK], DT, tag="t2")
            nc.vector.tensor_scalar(own[:], selg[:], meta[0:16, 1:2], None, op0=ALU.is_ge)
            nc.vector.tensor_scalar(t2[:], selg[:], meta[0:16, 3:4], None, op0=ALU.is_lt)
            nc.vector.tensor_tensor(out=own[:], in0=own[:], in1=t2[:], op=ALU.mult)
            nc.vector.tensor_tensor(out=alpha[:], in0=alpha[:], in1=own[:], op=ALU.mult)
            lidx = spool.tile([16, 16], DT, tag="lidx")
            nc.vector.memset(lidx[:], 0.0)
            nc.vector.tensor_scalar(lidx[:, 0:TOPK], selg[:], meta[0:16, 1:2], None, op0=ALU.subtract)
            nc.vector.tensor_scalar(lidx[:, 0:TOPK], lidx[:, 0:TOPK], 0.0, None, op0=ALU.max)
            nc.vector.tensor_scalar(lidx[:, 0:TOPK], lidx[:, 0:TOPK], float(NL - 1), None, op0=ALU.min)
            nc.vector.tensor_scalar(lidx[:, 0:TOPK], lidx[:, 0:TOPK], meta[0:16, 2:3], None, op0=ALU.add)
            # flat [256]: slot g = bh*16 + j (only j<10 used); gather tiles from halves
            ixdr2 = dpool.tile([1, 256], DT)
            nc.sync.dma_start(ixdr2[0, :].rearrange("(bh j) -> bh j", bh=16), lidx[:])
            gx2 = spool.tile([128, 2, F], DT, tag="gx")
            for b2 in range(2):
                ixl2 = spool.tile([128, 1], DT, tag=f"ixl{b2}")
                nc.sync.dma_start(ixl2[:], ixdr2[0, b2 * 128:(b2 + 1) * 128].rearrange("(p one) -> p one", one=1))
                ixi2 = spool.tile([128, 1], I32, tag=f"ixi{b2}")
                nc.vector.tensor_copy(ixi2[:], ixl2[:])
                nc.gpsimd.indirect_dma_start(
                    out=gx2[:, b2, :], out_offset=None,
                    in_=feat_in[:].rearrange("b n f -> (b n) f"),
                    in_offset=bass.IndirectOffsetOnAxis(ap=ixi2[:, 0:1], axis=0))
            # alpha padded to [16,16] -> flat [256] -> per-partition columns
            alphp = spool.tile([16, 16], DT, tag="alphp")
            nc.vector.memset(alphp[:], 0.0)
            nc.vector.tensor_copy(alphp[:, 0:TOPK], alpha[:])
            alphdr = dpool.tile([1, 256], DT)
            nc.sync.dma_start(alphdr[0, :].rearrange("(bh j) -> bh j", bh=16), alphp[:])
            alphav = spool.tile([128, 1], DT, tag="alphav")
            nc.sync.dma_start(alphav[:], alphdr[0, 0:128].rearrange("(p one) -> p one", one=1))
            alphav2 = spool.tile([128, 1], DT, tag="alphav2")
            nc.sync.dma_start(alphav2[:], alphdr[0, 128:256].rearrange("(p one) -> p one", one=1))
            ablk = spool.tile([128, 16], DT, tag="ablk")
            nc.vector.tensor_scalar(ablk[:], eqpA[:], alphav[:, 0:1], None, op0=ALU.mult)
            ablk2 = spool.tile([128, 16], DT, tag="ablk2")
            nc.vector.tensor_scalar(ablk2[:], eqpB[:], alphav2[:, 0:1], None, op0=ALU.mult)
            # corr[bh, f] = sum_j alpha_j x_j[f]
            corr = spool.tile([16, F], DT, tag="corr")
            for half in range(2):
                pc = psS.tile([16, RB], DT, tag="pS1")
                nc.tensor.matmul(pc[:], lhsT=ablk[:], rhs=gx2[:, 0, half * RB:(half + 1) * RB],
                                 start=True, stop=False)
                nc.tensor.matmul(pc[:], lhsT=ablk2[:], rhs=gx2[:, 1, half * RB:(half + 1) * RB],
                                 start=False, stop=True)
                nc.vector.tensor_copy(corr[:, half * RB:(half + 1) * RB], pc[:])
            # corrT[p, fc, bh]
            corrT = spool.tile([128, 8, 16], DT, tag="corrT")
            for fc in range(8):
                pt2 = ps.tile([128, 128], DT, tag="ptr", bufs=2)
                nc.tensor.transpose(pt2[0:128, 0:16], corr[:, fc * 128:(fc + 1) * 128],
                                    ident[0:16, 0:16])
                nc.scalar.activation(corrT[:, fc], pt2[0:128, 0:16], AF.Copy)
            # zinv broadcast to all partitions
            zdr = dpool.tile([1, 16], DT)
            nc.sync.dma_start(zdr[0, :].rearrange("(bh one) -> bh one", one=1), zinv[:])
            zrow = spool.tile([1, 16], DT, tag="zrow")
            nc.sync.dma_start(zrow[:], zdr[:])
            pz = ps.tile([128, 16], DT, tag="ptr", bufs=2)
            nc.tensor.matmul(pz[:], lhsT=onesr[:], rhs=zrow[:], start=True, stop=True)
            zrep = spool.tile([128, 16], DT, tag="zrep")
            nc.vector.tensor_copy(zrep[:], pz[:])
            # wT[p, h, fc, b] = (fsG[p, fc, b] + corrT[p, fc, b*8+h]) * zrep[p, b*8+h]
            wT = spool.tile([128, 8, 8, 2], DT, tag="wT")
            nc.vector.tensor_tensor(
                out=wT[:],
                in0=fsG[:].unsqueeze(1).broadcast_to((128, 8, 8, 2)),
                in1=corrT[:].rearrange("p fc (b h) -> p h fc b", b=2),
                op=ALU.add)
            nc.vector.tensor_tensor(
                out=wT[:],
                in0=wT[:],
                in1=zrep[:].rearrange("p (b h) -> p h b", b=2).unsqueeze(2).broadcast_to((128, 8, 8, 2)),
                op=ALU.mult)
            # agg = WcT^T @ wT (+bc)
            pagg = psS.tile([2, 128], DT, tag="pS1")
            for g in range(8):
                wcs = wpool.tile([128, 8, 128], DT, tag="wcc")
                nc.sync.dma_start(wcs[:], wct_in[g])
                for c8 in range(8):
                    ck = g * 8 + c8
                    nc.tensor.matmul(pagg[:], lhsT=wT[:, ck // 8, ck % 8], rhs=wcs[:, c8],
                                     start=(ck == 0), stop=(ck == 63))
            aggsb = spool.tile([2, 128], DT, tag="aggsb")
            nc.vector.tensor_tensor(out=aggsb[:], in0=pagg[:], in1=bcrep[:], op=ALU.add)
            nc.sync.dma_start(agg_out[:], aggsb[:])

            # ---------------- count output ----------------
            selb = spool.tile([128, TOPK], DT, tag="selb")
            for blk in range(8):
                nc.sync.dma_start(selb[blk::8, :], selg[:])
            iot = spool.tile([128, RB], I32, tag="iot")
            nc.gpsimd.iota(iot[:], pattern=[[1, RB]], base=0, channel_multiplier=0)
            iotf = spool.tile([128, RB], DT, tag="iotf")
            nc.vector.tensor_copy(iotf[:], iot[:])
            nc.vector.tensor_scalar(iotf[:], iotf[:], meta[:, 0:1], None, op0=ALU.add)
            macc = spool.tile([128, RB], DT, tag="macc")
            nc.vector.memset(macc[:], 0.0)
            eqm = spool.tile([128, RB], DT, tag="eqm")
            for j in range(TOPK):
                nc.vector.tensor_scalar(eqm[:], iotf[:], selb[:, j:j + 1], None, op0=ALU.is_equal)
                nc.vector.tensor_tensor(out=macc[:], in0=macc[:], in1=eqm[:], op=ALU.add)
            mtr = spool.tile([128, 4, 128], DT, tag="mtr")
            for q in range(4):
                ptr = ps.tile([128, 128], DT, tag="ptr", bufs=2)
                nc.tensor.transpose(ptr[:], macc[:, q * 128:(q + 1) * 128], ident[:])
                nc.scalar.activation(mtr[:, q], ptr[:], AF.Copy)
            mscr = dpool.tile([B, NL, H], DT)
            for b in range(2):
                for q in range(4):
                    for blk in range(8):
                        nc.sync.dma_start(
                            mscr[b].rearrange("(blk q2 n) h -> blk q2 n h", blk=8, q2=4)[blk, q],
                            mtr[:, q, b * 64 + blk: b * 64 + blk + 57: 8])
            cl = spool.tile([128, 64, 8], DT, tag="cl")
            nc.sync.dma_start(cl[:], cnt_in[:].rearrange("b (nb nl) h -> (b nb) nl h", nl=64))
            ml = spool.tile([128, 64, 8], DT, tag="ml")
            nc.sync.dma_start(ml[:], mscr[:].rearrange("b (nb nl) h -> (b nb) nl h", nl=64))
            nc.vector.tensor_tensor(out=cl[:], in0=cl[:], in1=ml[:], op=ALU.add)
            nc.sync.dma_start(cnt_out[:].rearrange("b (nb nl) h -> (b nb) nl h", nl=64), cl[:])

    _split_multi_waits(nc)
    return nc


def _host_prep(features, W1, b1, Wb, bb, Wc, bc, ucb_count, counter):
    features = np.ascontiguousarray(features, dtype=np.float32)
    W1 = np.asarray(W1, dtype=np.float32)
    b1 = np.asarray(b1, dtype=np.float32)
    Wb = np.asarray(Wb, dtype=np.float32)
    bb = np.asarray(bb, dtype=np.float32)
    Wc = np.asarray(Wc, dtype=np.float32)
    bc = np.asarray(bc, dtype=np.float32)
    ucb_count = np.asarray(ucb_count, dtype=np.float32)

    W1mat = W1.transpose(2, 0, 1).reshape(F, COLS)           # [f, col], col=h*HD+d
    W1m8 = W1mat.reshape(8, 128, COLS)
    import ml_dtypes
    W1mb = np.ascontiguousarray(W1m8).astype(ml_dtypes.bfloat16)
    # rescue layout: [h, p, k, d] = W1mat[k*128+p, h*256+d]
    W1m = np.ascontiguousarray(W1m8.reshape(8, 128, 8, 256).transpose(2, 1, 0, 3))
    b1rep = np.ascontiguousarray(np.tile(b1.reshape(1, COLS), (16, 1)))
    Wbrep_row = np.zeros((COLS,), np.float32)
    for h in range(H):
        Wbrep_row[h * HD:(h + 1) * HD] = Wb[h, 0, :]
    Wbrep = np.ascontiguousarray(np.tile(Wbrep_row.reshape(1, COLS), (16, 1)))
    b1flat = b1.reshape(COLS)
    b1T = np.ascontiguousarray(b1flat.reshape(16, 128).T)
    Wbig = np.zeros((COLS, 8), np.float32)
    for h in range(H):
        Wbig[h * HD:(h + 1) * HD, h] = Wb[h, 0, :]
    Wbg = np.ascontiguousarray(Wbig.reshape(16, 128, 8))
    Wbm = np.ascontiguousarray(Wb[:, 0, :].reshape(8, 2, 128).transpose(2, 1, 0))
    bbv = np.ascontiguousarray(bb.reshape(8, 1))
    bb16 = np.ascontiguousarray(np.tile(bb.reshape(1, 8), (2, 1)).reshape(16, 1))

    # ucb constant per (b,h): replicate reference fp32 ops
    Ct = ucb_count.transpose(0, 3, 2, 1)                     # (B,H,R,N)
    ssum = Ct.sum(axis=-1, dtype=np.float32) + np.float32(1e-6)   # (B,H,R)
    log_iter = np.float32(np.log(max(1, int(counter))))
    cub = np.sqrt((log_iter / ssum).astype(np.float32)).astype(np.float32)  # BETA=1
    cvec = np.ascontiguousarray(cub.reshape(16, 1))

    eqpA = np.zeros((128, 16), np.float32)
    eqpB = np.zeros((128, 16), np.float32)
    for p in range(128):
        eqpA[p, p // 16] = 1.0
        if (p + 128) // 16 < 16:
            eqpB[p, (p + 128) // 16] = 1.0

    shared = dict(W1m=W1m, W1mb=W1mb, b1rep=b1rep, Wbrep=Wbrep, b1T=b1T,
                  Wbg=Wbg, Wbm=Wbm, bbv=bbv, bb16=bb16,
                  cvec=cvec, eqpA=eqpA, eqpB=eqpB)

    in_maps = []
    for c in range(NCORES):
        fshard = np.ascontiguousarray(features[:, c * NL:(c + 1) * NL, :])
        cshard = np.ascontiguousarray(ucb_count[:, c * NL:(c + 1) * NL, 0, :])
        Wcs = Wc[c * 128:(c + 1) * 128, :]                   # [128 fo, 8192]
        WcT = np.ascontiguousarray(
            Wcs.T.reshape(8, 8, 128, 128).transpose(0, 2, 1, 3).reshape(8, 128, 1024))
        bcs = np.ascontiguousarray(bc[c * 128:(c + 1) * 128].reshape(128, 1))
        bcrep = np.ascontiguousarray(np.tile(bc[c * 128:(c + 1) * 128].reshape(1, 128), (2, 1)))
        meta = np.zeros((128, 4), np.float32)
        for p in range(128):
            meta[p, 0] = c * NL + (p % 8) * RB               # rowbase for mask iota
            meta[p, 1] = c * NL                              # coreoff
            meta[p, 3] = c * NL + NL
        for p in range(16):
            meta[p, 2] = (p // 8) * NL                       # b*NL for [16,x] rows
        m = dict(shared)
        m.update(feat=fshard, cnt=cshard, WcT=WcT, bcs=bcs, bcrep=bcrep, meta=meta)
        in_maps.append(m)
    return in_maps


def kernel(features, W1, b1, Wb, bb, Wc, bc, ucb_count, counter):
    if "nc" not in _CACHE:
        _CACHE["nc"] = _build()
    nc = _CACHE["nc"]
    in_maps = _host_prep(features, W1, b1, Wb, bb, Wc, bc, ucb_count, counter)
    res = run_bass_kernel_spmd(nc, in_maps, list(range(NCORES)))
    rs = res.results

    head_attentions = np.empty((B, 1, H, N), np.float32)
    for c in range(NCORES):
        s = rs[c]["S_out"]                                   # [16, NL]
        head_attentions[:, 0, :, c * NL:(c + 1) * NL] = s.reshape(2, 8, NL)
    new_count = np.empty((B, N, 1, H), np.float32)
    for c in range(NCORES):
        new_count[:, c * NL:(c + 1) * NL, 0, :] = rs[c]["cnt_out"]
    agg = np.empty((B, 1, F), np.float32)
    for c in range(NCORES):
        agg[:, 0, c * 128:(c + 1) * 128] = rs[c]["agg_out"]  # [2, 128 fo]
    return agg, head_attentions, new_count


# revision 23
# speedup vs baseline: 1.7480x; 1.7480x over previous
"""Trainium2 Bass kernel for nn_ABMIL_UCB (topk_masking).

8-core SPMD, data-parallel over N (patches). Per core:
  phase 1: stream the 32MB feature shard once; PE-transpose tiles; fp32r
           matmuls X@W1^T -> tanh(+b1) -> Wb-dot => fast S [16bh, 4096];
           featsum accumulated by the ACT transpose-copies (accum_out).
  phase 2: local top-16 candidates per (b,h) via DVE max8/max_index;
           exact-fp32 rescue recompute of candidate scores (dma_gather rows,
           fp32 matmuls + ACT tanh); ONE AllGather of
           (exact candidate S, global idx, featsum partials).
  phase 3 (replicated on all cores): global exact top-10 per (b,h) using the
           reference's ucb (+const per (b,h)) fp32 bucketing and lowest-index
           tie-breaks; masked-softmax pooling via the algebraic form
           weighted = (featsum + sum_j (e^{s~_j}-1) x_j) / (N-K+sum_j e^{s~_j});
           agg = Wc @ weighted (fo-sharded across cores); one-hot count update.
Host only shards/reshapes inputs and reassembles outputs.
"""

import sys
import os

for _p in ("/opt/trn_rl_repo", "/root/.axon_site/_ro/trn_rl_repo"):
    if os.path.isdir(_p) and _p not in sys.path:
        sys.path.append(_p)

import numpy as np

from concourse import bass, mybir, masks
from concourse import tile
from concourse.bass_utils import run_bass_kernel_spmd

DT = mybir.dt.float32
FR = mybir.dt.float32r
I16 = mybir.dt.int16
I32 = mybir.dt.int32
U32 = mybir.dt.uint32
BF = mybir.dt.bfloat16
AF = mybir.ActivationFunctionType
ALU = mybir.AluOpType
AX = mybir.AxisListType

B, N, F, HD, H = 2, 32768, 1024, 256, 8
NCORES = 8
NL = N // NCORES            # 4096 patches per core
COLS = H * HD               # 2048
NBLK = 16                   # row blocks of 512 (8 per b)
RB = 512
TOPK = 10
CAND = 16                   # local candidates per (b, h)
NEG = -1.0e30
INF = 3.0e38

_CACHE = {}


def _split_multi_waits(nc, max_waits=1):
    """This neuronxcc build accepts only one sync-wait per instruction;
    move extra waits onto preceding wait-only NoOps."""
    n = 0
    for bb in nc.main_func.blocks:
        insts = bb.instructions
        new = []
        changed = False
        for ins in insts:
            si = getattr(ins, "sync_info", None)
            if si is not None and si.on_wait and len(si.on_wait) > max_waits:
                waits = list(si.on_wait)
                for w in waits[:-max_waits]:
                    nop = mybir.InstNoOp(
                        name=f"I-waitsplit-{nc.next_id()}",
                        sync_info=mybir.SyncInfo(on_wait=[w], on_update=[]),
                        bass_nofuse=True,
                        engine=ins.engine,
                    )
                    nc.register_instruction(nop, overwrite=True)
                    new.append(nop)
                    n += 1
                ins.sync_info = mybir.SyncInfo(
                    on_wait=waits[-max_waits:],
                    on_update=list(si.on_update) if si.on_update else [],
                )
                changed = True
            new.append(ins)
        if changed:
            bb.instructions = new
    return n


def _build():
    core_ids = list(range(NCORES))
    nc = bass.Bass()

    feat_in = nc.declare_dram_parameter("feat", [B, NL, F], DT, isOutput=False)
    w1_in = nc.declare_dram_parameter("W1m", [8, 128, 8, 256], DT, isOutput=False)
    w1b_in = nc.declare_dram_parameter("W1mb", [8, 128, COLS], BF, isOutput=False)
    b1r_in = nc.declare_dram_parameter("b1rep", [16, COLS], DT, isOutput=False)
    wbr_in = nc.declare_dram_parameter("Wbrep", [16, COLS], DT, isOutput=False)
    bcr_in = nc.declare_dram_parameter("bcrep", [2, 128], DT, isOutput=False)
    b1_in = nc.declare_dram_parameter("b1T", [128, 16], DT, isOutput=False)
    wbg_in = nc.declare_dram_parameter("Wbg", [16, 128, 8], DT, isOutput=False)
    wbm_in = nc.declare_dram_parameter("Wbm", [128, 2, 8], DT, isOutput=False)
    bbv_in = nc.declare_dram_parameter("bbv", [8, 1], DT, isOutput=False)
    bb16_in = nc.declare_dram_parameter("bb16", [16, 1], DT, isOutput=False)
    wct_in = nc.declare_dram_parameter("WcT", [8, 128, 1024], DT, isOutput=False)
    bcs_in = nc.declare_dram_parameter("bcs", [128, 1], DT, isOutput=False)
    cnt_in = nc.declare_dram_parameter("cnt", [B, NL, H], DT, isOutput=False)
    cvec_in = nc.declare_dram_parameter("cvec", [16, 1], DT, isOutput=False)
    # meta[:,0]=rowbase (core*NL + (p%8)*512); meta[:,1]=coreoff (core*NL);
    # meta[:,2]= (p<16): (p//8)*NL else 0; meta[:,3]=coreoff+NL
    meta_in = nc.declare_dram_parameter("meta", [128, 4], DT, isOutput=False)
    eqpa_in = nc.declare_dram_parameter("eqpA", [128, 16], DT, isOutput=False)
    eqpb_in = nc.declare_dram_parameter("eqpB", [128, 16], DT, isOutput=False)

    s_out = nc.declare_dram_parameter("S_out", [16, NL], DT, isOutput=True)
    cnt_out = nc.declare_dram_parameter("cnt_out", [B, NL, H], DT, isOutput=True)
    agg_out = nc.declare_dram_parameter("agg_out", [2, 128], DT, isOutput=True)

    with tile.TileContext(nc) as tc:
        with tc.tile_pool(name="const", bufs=1) as cpool, \
             tc.tile_pool(name="work", bufs=2) as wpool, \
             tc.tile_pool(name="nat", bufs=2) as npool, \
             tc.tile_pool(name="avp", bufs=3) as apool, \
             tc.tile_pool(name="small", bufs=1) as spool, \
             tc.tile_pool(name="ps", bufs=3, space="PSUM") as ps, \
             tc.tile_pool(name="psS", bufs=1, space="PSUM") as psS, \
             tc.tile_pool(name="dram", bufs=1, space="DRAM") as dpool:

            ident = cpool.tile([128, 128], DT, tag="ident")
            masks.make_identity(nc, ident[:])

            identB = cpool.tile([128, 128], BF, tag="identB")
            masks.make_identity(nc, identB[:])
            w1 = cpool.tile([128, 8, COLS], BF, tag="w1")
            for k in range(8):
                nc.sync.dma_start(w1[:, k], w1b_in[k])
            b1rep = cpool.tile([16, COLS], DT, tag="b1rep")
            nc.sync.dma_start(b1rep[:], b1r_in[:])
            wbrep = cpool.tile([16, COLS], DT, tag="wbrep")
            nc.sync.dma_start(wbrep[:], wbr_in[:])
            bcrep = cpool.tile([2, 128], DT, tag="bcrep")
            nc.sync.dma_start(bcrep[:], bcr_in[:])
            wbig = cpool.tile([128, 16, 8], FR, tag="wbig")
            for m in range(16):
                nc.gpsimd.dma_start(wbig[:, m], wbg_in[m])
            b1T = cpool.tile([128, 16], DT, tag="b1T")
            nc.sync.dma_start(b1T[:], b1_in[:])
            wbm = cpool.tile([128, 2, 8], DT, tag="wbm")
            nc.sync.dma_start(wbm[:], wbm_in[:])
            bbv = cpool.tile([8, 1], DT, tag="bbv")
            nc.sync.dma_start(bbv[:], bbv_in[:])
            bb16 = cpool.tile([16, 1], DT, tag="bb16")
            nc.sync.dma_start(bb16[:], bb16_in[:])
            bcs = cpool.tile([128, 1], DT, tag="bcs")
            nc.sync.dma_start(bcs[:], bcs_in[:])
            cvec = cpool.tile([16, 1], DT, tag="cvec")
            nc.sync.dma_start(cvec[:], cvec_in[:])
            meta = cpool.tile([128, 4], DT, tag="meta")
            nc.sync.dma_start(meta[:], meta_in[:])
            eqpA = cpool.tile([128, 16], DT, tag="eqpA")
            nc.sync.dma_start(eqpA[:], eqpa_in[:])
            eqpB = cpool.tile([128, 16], DT, tag="eqpB")
            nc.sync.dma_start(eqpB[:], eqpb_in[:])
            onesr = cpool.tile([1, 128], DT, tag="onesr")
            nc.vector.memset(onesr[:], 1.0)

            accfs = spool.tile([128, NBLK, 8, 4], DT, tag="accfs")
            sdr = dpool.tile([16, NL], DT)

            # ---------------- phase 1 ----------------
            # per-block transpose emitters so transposes interleave with the
            # previous block's matmuls (keeps the PE HAM clock warm)
            def make_xtr_ops(r):
                b, rb = r // 8, r % 8
                Xtr = wpool.tile([128, 8, RB], BF, tag="Xtr", name=f"Xtr{r}")
                ops = []
                for t in range(4):
                    natX = npool.tile([128, F], DT, tag="natX", name=f"natX{r}_{t}")
                    nc.sync.dma_start(
                        natX[:], feat_in[b, rb * RB + t * 128: rb * RB + (t + 1) * 128, :])
                    natB = npool.tile([128, F], BF, tag="natB", name=f"natB{r}_{t}")
                    nc.vector.tensor_copy(natB[:], natX[:])

                    def tr_op(t=t, k=None, natB=natB, Xtr=Xtr, r=r):
                        for k in range(8):
                            ptr = ps.tile([128, 128], BF, tag="ptr", bufs=2,
                                          name=f"ptr{r}_{t}_{k}")
                            nc.tensor.transpose(ptr[:], natB[:, k * 128:(k + 1) * 128], identB[:])
                            nc.scalar.activation(
                                Xtr[:, k, t * 128:(t + 1) * 128], ptr[:], AF.Copy,
                                accum_out=accfs[:, r, k, t:t + 1])
                    ops.append(tr_op)
                return Xtr, ops

            pend = None  # delayed S-matmul: (m, avt, pS)
            Xtr_cur, ops0 = make_xtr_ops(0)
            for op in ops0:
                op()
            for r in range(NBLK):
                b, rb = r // 8, r % 8
                if r + 1 < NBLK:
                    Xtr_next, next_ops = make_xtr_ops(r + 1)
                else:
                    Xtr_next, next_ops = None, []
                pS = psS.tile([8, RB], DT, tag=f"pS{r % 2}", name=f"pS{r}")
                for m in range(16):
                    pt = ps.tile([128, RB], DT, tag="pt", name=f"pt{r}_{m}")
                    for k in range(8):
                        nc.tensor.matmul(pt[:], lhsT=w1[:, k, m * 128:(m + 1) * 128],
                                         rhs=Xtr_cur[:, k], start=(k == 0), stop=(k == 7))
                    if pend is not None:
                        pm, pavt, ppS = pend
                        nc.tensor.matmul(ppS[:], lhsT=wbig[:, pm], rhs=pavt[:],
                                         start=(pm == 0), stop=(pm == 15))
                    # sprinkle next block's transposes between m-tiles
                    if m % 4 == 1 and next_ops:
                        next_ops.pop(0)()
                    avt = apool.tile([128, RB], FR, tag="avt", name=f"avt{r}_{m}")
                    nc.scalar.activation(avt[:], pt[:], AF.Tanh, bias=b1T[:, m:m + 1])
                    pend = (m, avt, pS)
                pm, pavt, ppS = pend
                nc.tensor.matmul(ppS[:], lhsT=wbig[:, pm], rhs=pavt[:],
                                 start=(pm == 0), stop=(pm == 15))
                pend = None
                for op in next_ops:
                    op()
                sstage = apool.tile([8, RB], DT, tag="sstage", name=f"sstage{r}")
                nc.vector.tensor_scalar(sstage[:], pS[:], bbv[:, 0:1], None, op0=ALU.add)
                nc.sync.dma_start(sdr[b * 8:(b + 1) * 8, rb * RB:(rb + 1) * RB], sstage[:])
                Xtr_cur = Xtr_next

            # featsum partials: fs[p, k, b]
            fs = spool.tile([128, 8, 2], DT, tag="fs")
            nc.vector.tensor_reduce(
                fs[:], accfs[:].rearrange("p (b blk) k t -> p k b blk t", b=2),
                axis=AX.XY, op=ALU.add)

            # ---------------- phase 2: local candidates + rescue ----------------
            nc.sync.dma_start(s_out[:], sdr[:])
            wk = spool.tile([16, NL], DT, tag="wkt")
            nc.sync.dma_start(wk[:], sdr[:])
            mx = spool.tile([16, CAND], DT, tag="mx")
            ixu = spool.tile([16, CAND], U32, tag="ixu")
            for r0 in range(CAND // 8):
                nc.vector.max(mx[:, r0 * 8:(r0 + 1) * 8], wk[:])
                nc.vector.max_index(ixu[:, r0 * 8:(r0 + 1) * 8], mx[:, r0 * 8:(r0 + 1) * 8], wk[:])
                if r0 + 1 < CAND // 8:
                    nc.vector.match_replace(wk[:], mx[:, r0 * 8:(r0 + 1) * 8], wk[:], NEG)

            ixf = spool.tile([16, CAND], DT, tag="ixf")
            nc.vector.tensor_copy(ixf[:], ixu[:])
            gidx = spool.tile([16, CAND], DT, tag="gidx")
            nc.vector.tensor_scalar(gidx[:], ixf[:], meta[0:16, 1:2], None, op0=ALU.add)
            # row index into flattened [B*NL, F]: + b*NL
            ixrow = spool.tile([16, CAND], DT, tag="ixrow")
            nc.vector.tensor_scalar(ixrow[:], ixf[:], meta[0:16, 2:3], None, op0=ALU.add)
            # gather slot g = b*128 + (h*16+i): flat DRAM roundtrip gives per-b
            # [128,1] per-partition index tiles
            ixdr = dpool.tile([1, 256], DT)
            nc.sync.dma_start(ixdr[0, :].rearrange("(bh i) -> bh i", bh=16), ixrow[:])
            gx = spool.tile([128, 2, F], DT, tag="gx")
            for b2 in range(2):
                ixl = spool.tile([128, 1], DT, tag=f"ixl{b2}")
                nc.sync.dma_start(ixl[:], ixdr[0, b2 * 128:(b2 + 1) * 128].rearrange("(p one) -> p one", one=1))
                ixi = spool.tile([128, 1], I32, tag=f"ixi{b2}")
                nc.vector.tensor_copy(ixi[:], ixl[:])
                nc.gpsimd.indirect_dma_start(
                    out=gx[:, b2, :], out_offset=None,
                    in_=feat_in[:].rearrange("b n f -> (b n) f"),
                    in_offset=bass.IndirectOffsetOnAxis(ap=ixi[:, 0:1], axis=0))
            # transpose gathered rows -> XcT[p=f%128, (c2b, k), 128 rows]
            XcT = spool.tile([128, 2, 8, 128], DT, tag="XcT")
            for c2 in range(2):
                for k in range(8):
                    ptr = ps.tile([128, 128], DT, tag="ptr", bufs=2)
                    nc.tensor.transpose(ptr[:], gx[:, c2, k * 128:(k + 1) * 128], ident[:])
                    nc.scalar.activation(XcT[:, c2, k], ptr[:], AF.Copy)
            # exact per-head S for candidates: av^T-cand[16 rows, 256 d] via
            # stationary-XcT matmuls (tiny LDWEIGHTS), Wb-dot on DVE
            sxdr = dpool.tile([16, CAND], DT)
            for h in range(8):
                w1h = wpool.tile([128, 8, 256], DT, tag="w1h", bufs=2)
                nc.sync.dma_start(w1h[:], w1_in[h])
                for b2 in range(2):
                    pa = ps.tile([16, 256], DT, tag="pa", bufs=1)
                    for k in range(8):
                        nc.tensor.matmul(
                            pa[:], lhsT=XcT[:, b2, k, h * 16:(h + 1) * 16],
                            rhs=w1h[:, k], start=(k == 0), stop=(k == 7))
                    avc = apool.tile([16, 256], DT, tag="avc")
                    nc.vector.tensor_tensor(out=avc[:], in0=pa[:],
                                            in1=b1rep[:, h * 256:(h + 1) * 256], op=ALU.add)
                    nc.scalar.activation(avc[:], avc[:], AF.Tanh)
                    nc.vector.tensor_tensor(out=avc[:], in0=avc[:],
                                            in1=wbrep[:, h * 256:(h + 1) * 256], op=ALU.mult)
                    scand = apool.tile([16, 1], DT, tag="scand")
                    nc.vector.tensor_reduce(scand[:], avc[:], axis=AX.X, op=ALU.add)
                    nc.sync.dma_start(
                        sxdr[b2 * 8 + h, :].rearrange("(i one) -> i one", one=1), scand[:])
            Sx = spool.tile([16, CAND], DT, tag="Sx")
            nc.sync.dma_start(Sx[:], sxdr[:])
            nc.vector.tensor_scalar(Sx[:], Sx[:], bb16[:, 0:1], None, op0=ALU.add)

            # ---------------- AllGather ----------------
            PAY = 16 * CAND * 2 + 2048       # 2560 fp32
            agin = dpool.tile([1, PAY], DT)
            agout = dpool.tile([NCORES, PAY], DT)
            nc.sync.dma_start(agin[0, 0:256].rearrange("(bh i) -> bh i", bh=16), Sx[:])
            nc.sync.dma_start(agin[0, 256:512].rearrange("(bh i) -> bh i", bh=16), gidx[:])
            for b in range(2):
                nc.sync.dma_start(
                    agin[0, 512 + b * 1024: 512 + (b + 1) * 1024].rearrange("(k p) -> p k", k=8),
                    fs[:, :, b])
            nc.gpsimd.collective_compute(
                "AllGather", ALU.bypass, replica_groups=[core_ids],
                ins=[agin.opt()], outs=[agout.opt()])

            NCAND = NCORES * CAND  # 128
            SallC = spool.tile([16, NCAND], DT, tag="SallC")
            nc.sync.dma_start(
                SallC[:].rearrange("bh (c i) -> bh c i", c=NCORES),
                agout[:, 0:256].rearrange("c (bh i) -> bh c i", bh=16))
            Gall = spool.tile([16, NCAND], DT, tag="Gall")
            nc.sync.dma_start(
                Gall[:].rearrange("bh (c i) -> bh c i", c=NCORES),
                agout[:, 256:512].rearrange("c (bh i) -> bh c i", bh=16))
            fsnat = spool.tile([8, 2048], DT, tag="fsnat")
            nc.sync.dma_start(fsnat[:], agout[:, 512:2560])
            ones8 = spool.tile([8, 1], DT, tag="ones8")
            nc.vector.memset(ones8[:], 1.0)
            pfs = psS.tile([128, 16], DT, tag="pS0")
            for kb in range(16):
                nc.tensor.matmul(pfs[:, kb:kb + 1], lhsT=fsnat[:, kb * 128:(kb + 1) * 128],
                                 rhs=ones8[:], start=True, stop=True)
            # pfs[p, kb] with kb = b*8+k ordering from payload (b k p)
            fsG = spool.tile([128, 8, 2], DT, tag="fsG")
            nc.vector.tensor_copy(fsG[:].rearrange("p k b -> p b k"),
                                  pfs[:].rearrange("p (b k) -> p b k", b=2))

            # ---------------- phase 3: global select (replicated) ----------------
            ucb = spool.tile([16, NCAND], DT, tag="ucb")
            nc.vector.tensor_scalar(ucb[:], SallC[:], cvec[:, 0:1], None, op0=ALU.add)
            inft = spool.tile([16, NCAND], DT, tag="inft")
            nc.vector.memset(inft[:], INF)
            ninft = spool.tile([16, NCAND], DT, tag="ninft")
            nc.vector.memset(ninft[:], NEG)
            selv = spool.tile([16, TOPK], DT, tag="selv")
            selg = spool.tile([16, TOPK], DT, tag="selg")
            mxv = spool.tile([16, 1], DT, tag="mxv")
            gmin = spool.tile([16, 1], DT, tag="gmin")
            eqv = spool.tile([16, NCAND], mybir.dt.uint8, tag="eqv")
            tmpm = spool.tile([16, NCAND], DT, tag="tmpm")
            for j in range(TOPK):
                nc.vector.tensor_reduce(mxv[:], ucb[:], axis=AX.X, op=ALU.max)
                nc.vector.tensor_scalar(eqv[:], ucb[:], mxv[:, 0:1], None, op0=ALU.is_ge)
                nc.vector.select(tmpm[:], eqv[:], Gall[:], inft[:])
                nc.vector.tensor_reduce(gmin[:], tmpm[:], axis=AX.X, op=ALU.min)
                nc.vector.tensor_copy(selg[:, j:j + 1], gmin[:])
                nc.vector.tensor_scalar(eqv[:], Gall[:], gmin[:, 0:1], None, op0=ALU.is_equal)
                nc.vector.select(tmpm[:], eqv[:], SallC[:], ninft[:])
                nc.vector.tensor_reduce(selv[:, j:j + 1], tmpm[:], axis=AX.X, op=ALU.max)
                nc.vector.copy_predicated(ucb[:], eqv[:], ninft[:])

            # pooling scalars
            ssum = spool.tile([16, 1], DT, tag="ssum")
            nc.vector.tensor_reduce(ssum[:], selv[:], axis=AX.X, op=ALU.add)
            nc.vector.tensor_scalar(ssum[:], ssum[:], 1e-6, None, op0=ALU.add)
            rs = spool.tile([16, 1], DT, tag="rs")
            nc.vector.reciprocal(rs[:], ssum[:])
            stl = spool.tile([16, TOPK], DT, tag="stl")
            nc.vector.tensor_scalar(stl[:], selv[:], rs[:, 0:1], None, op0=ALU.mult)
            exv = spool.tile([16, TOPK], DT, tag="exv")
            nc.scalar.activation(exv[:], stl[:], AF.Exp)
            zs = spool.tile([16, 1], DT, tag="zs")
            nc.vector.tensor_reduce(zs[:], exv[:], axis=AX.X, op=ALU.add)
            nc.vector.tensor_scalar(zs[:], zs[:], float(N - TOPK), None, op0=ALU.add)
            zinv = spool.tile([16, 1], DT, tag="zinv")
            nc.vector.reciprocal(zinv[:], zs[:])
            alpha = spool.tile([16, TOPK], DT, tag="alpha")
            nc.vector.tensor_scalar(alpha[:], exv[:], -1.0, None, op0=ALU.add)
            nc.vector.tensor_scalar(alpha[:], alpha[:], zinv[:, 0:1], None, op0=ALU.mult)
            # ownership mask and local row indices
            own = spool.tile([16, TOPK], DT, tag="own")
            t2 = spool.tile([16, TOPK], DT, tag="t2")
            nc.vector.tensor_scalar(own[:], selg[:], meta[0:16, 1:2], None, op0=ALU.is_ge)
            nc.vector.tensor_scalar(t2[:], selg[:], meta[0:16, 3:4], None, op0=ALU.is_lt)
            nc.vector.tensor_tensor(out=own[:], in0=own[:], in1=t2[:], op=ALU.mult)
            nc.vector.tensor_tensor(out=alpha[:], in0=alpha[:], in1=own[:], op=ALU.mult)
            lidx = spool.tile([16, 16], DT, tag="lidx")
            nc.vector.memset(lidx[:], 0.0)
            nc.vector.tensor_scalar(lidx[:, 0:TOPK], selg[:], meta[0:16, 1:2], None, op0=ALU.subtract)
            nc.vector.tensor_scalar(lidx[:, 0:TOPK], lidx[:, 0:TOPK], 0.0, None, op0=ALU.max)
            nc.vector.tensor_scalar(lidx[:, 0:TOPK], lidx[:, 0:TOPK], float(NL - 1), None, op0=ALU.min)
            nc.vector.tensor_scalar(lidx[:, 0:TOPK], lidx[:, 0:TOPK], meta[0:16, 2:3], None, op0=ALU.add)
            # flat [256]: slot g = bh*16 + j (only j<10 used); gather tiles from halves
            ixdr2 = dpool.tile([1, 256], DT)
            nc.sync.dma_start(ixdr2[0, :].rearrange("(bh j) -> bh j", bh=16), lidx[:])
            gx2 = spool.tile([128, 2, F], DT, tag="gx")
            for b2 in range(2):
                ixl2 = spool.tile([128, 1], DT, tag=f"ixl{b2}")
                nc.sync.dma_start(ixl2[:], ixdr2[0, b2 * 128:(b2 + 1) * 128].rearrange("(p one) -> p one", one=1))
                ixi2 = spool.tile([128, 1], I32, tag=f"ixi{b2}")
                nc.vector.tensor_copy(ixi2[:], ixl2[:])
                nc.gpsimd.indirect_dma_start(
                    out=gx2[:, b2, :], out_offset=None,
                    in_=feat_in[:].rearrange("b n f -> (b n) f"),
                    in_offset=bass.IndirectOffsetOnAxis(ap=ixi2[:, 0:1], axis=0))
            # alpha padded to [16,16] -> flat [256] -> per-partition columns
            alphp = spool.tile([16, 16], DT, tag="alphp")
            nc.vector.memset(alphp[:], 0.0)
            nc.vector.tensor_copy(alphp[:, 0:TOPK], alpha[:])
            alphdr = dpool.tile([1, 256], DT)
            nc.sync.dma_start(alphdr[0, :].rearrange("(bh j) -> bh j", bh=16), alphp[:])
            alphav = spool.tile([128, 1], DT, tag="alphav")
            nc.sync.dma_start(alphav[:], alphdr[0, 0:128].rearrange("(p one) -> p one", one=1))
            alphav2 = spool.tile([128, 1], DT, tag="alphav2")
            nc.sync.dma_start(alphav2[:], alphdr[0, 128:256].rearrange("(p one) -> p one", one=1))
            ablk = spool.tile([128, 16], DT, tag="ablk")
            nc.vector.tensor_scalar(ablk[:], eqpA[:], alphav[:, 0:1], None, op0=ALU.mult)
            ablk2 = spool.tile([128, 16], DT, tag="ablk2")
            nc.vector.tensor_scalar(ablk2[:], eqpB[:], alphav2[:, 0:1], None, op0=ALU.mult)
            # corr[bh, f] = sum_j alpha_j x_j[f]
            corr = spool.tile([16, F], DT, tag="corr")
            for half in range(2):
                pc = psS.tile([16, RB], DT, tag="pS1")
                nc.tensor.matmul(pc[:], lhsT=ablk[:], rhs=gx2[:, 0, half * RB:(half + 1) * RB],
                                 start=True, stop=False)
                nc.tensor.matmul(pc[:], lhsT=ablk2[:], rhs=gx2[:, 1, half * RB:(half + 1) * RB],
                                 start=False, stop=True)
                nc.vector.tensor_copy(corr[:, half * RB:(half + 1) * RB], pc[:])
            # corrT[p, fc, bh]
            corrT = spool.tile([128, 8, 16], DT, tag="corrT")
            for fc in range(8):
                pt2 = ps.tile([128, 128], DT, tag="ptr", bufs=2)
                nc.tensor.transpose(pt2[0:128, 0:16], corr[:, fc * 128:(fc + 1) * 128],
                                    ident[0:16, 0:16])
                nc.scalar.activation(corrT[:, fc], pt2[0:128, 0:16], AF.Copy)
            # zinv broadcast to all partitions
            zdr = dpool.tile([1, 16], DT)
            nc.sync.dma_start(zdr[0, :].rearrange("(bh one) -> bh one", one=1), zinv[:])
            zrow = spool.tile([1, 16], DT, tag="zrow")
            nc.sync.dma_start(zrow[:], zdr[:])
            pz = ps.tile([128, 16], DT, tag="ptr", bufs=2)
            nc.tensor.matmul(pz[:], lhsT=onesr[:], rhs=zrow[:], start=True, stop=True)
            zrep = spool.tile([128, 16], DT, tag="zrep")
            nc.vector.tensor_copy(zrep[:], pz[:])
            # wT[p, h, fc, b] = (fsG[p, fc, b] + corrT[p, fc, b*8+h]) * zrep[p, b*8+h]
            wT = spool.tile([128, 8, 8, 2], DT, tag="wT")
            nc.vector.tensor_tensor(
                out=wT[:],
                in0=fsG[:].unsqueeze(1).broadcast_to((128, 8, 8, 2)),
                in1=corrT[:].rearrange("p fc (b h) -> p h fc b", b=2),
                op=ALU.add)
            nc.vector.tensor_tensor(
                out=wT[:],
                in0=wT[:],
                in1=zrep[:].rearrange("p (b h) -> p h b", b=2).unsqueeze(2).broadcast_to((128, 8, 8, 2)),
                op=ALU.mult)
            # agg = WcT^T @ wT (+bc)
            pagg = psS.tile([2, 128], DT, tag="pS1")
            for g in range(8):
                wcs = wpool.tile([128, 8, 128], DT, tag="wcc")
                nc.sync.dma_start(wcs[:], wct_in[g])
                for c8 in range(8):
                    ck = g * 8 + c8
                    nc.tensor.matmul(pagg[:], lhsT=wT[:, ck // 8, ck % 8], rhs=wcs[:, c8],
                                     start=(ck == 0), stop=(ck == 63))
            aggsb = spool.tile([2, 128], DT, tag="aggsb")
            nc.vector.tensor_tensor(out=aggsb[:], in0=pagg[:], in1=bcrep[:], op=ALU.add)
            nc.sync.dma_start(agg_out[:], aggsb[:])

            # ---------------- count output ----------------
            # mask built directly in the count's natural layout [p=(b,nb), nl, h]:
            # selrep[p, j, h] = selg[b(p)*8+h, j] via K=1 ones-matmul broadcast,
            # then 10 iota-compare accumulations; add counts in SBUF.
            selgdr = dpool.tile([16, TOPK], DT)
            nc.sync.dma_start(selgdr[:], selg[:])
            selrow = spool.tile([1, 2, 80], DT, tag="selrow")
            for b in range(2):
                nc.sync.dma_start(
                    selrow[0:1, b, :].rearrange("one (j h) -> one j h", j=TOPK),
                    selgdr[b * 8:(b + 1) * 8, :].rearrange("h j -> j h").unsqueeze(0))
            psel = ps.tile([128, 80], DT, tag="ptr", bufs=2)
            nc.tensor.matmul(psel[0:64, :], lhsT=onesr[0:1, 0:64], rhs=selrow[0:1, 0, :],
                             start=True, stop=True)
            nc.tensor.matmul(psel[64:128, :], lhsT=onesr[0:1, 0:64], rhs=selrow[0:1, 1, :],
                             start=True, stop=True)
            selrep = spool.tile([128, TOPK, 8], DT, tag="selrep")
            nc.scalar.activation(selrep[:], psel[:], AF.Copy)
            iotn = spool.tile([128, 64], I32, tag="iotn")
            nc.gpsimd.iota(iotn[:], pattern=[[1, 64]], base=0, channel_multiplier=0)
            iotnf = spool.tile([128, 64], DT, tag="iotnf")
            nc.vector.tensor_copy(iotnf[:], iotn[:])
            nc.vector.tensor_scalar(iotnf[:], iotnf[:], meta[:, 0:1], None, op0=ALU.add)
            maccn = spool.tile([128, 64, 8], DT, tag="maccn")
            nc.vector.memset(maccn[:], 0.0)
            eqn = spool.tile([128, 64, 8], DT, tag="eqn")
            for j in range(TOPK):
                nc.vector.tensor_tensor(
                    out=eqn[:],
                    in0=iotnf[:].unsqueeze(2).broadcast_to((128, 64, 8)),
                    in1=selrep[:, j, :].unsqueeze(1).broadcast_to((128, 64, 8)),
                    op=ALU.is_equal)
                nc.vector.tensor_tensor(out=maccn[:], in0=maccn[:], in1=eqn[:], op=ALU.add)
            cl = spool.tile([128, 64, 8], DT, tag="cl")
            nc.sync.dma_start(cl[:], cnt_in[:].rearrange("b (nb nl) h -> (b nb) nl h", nl=64))
            nc.vector.tensor_tensor(out=cl[:], in0=cl[:], in1=maccn[:], op=ALU.add)
            nc.sync.dma_start(cnt_out[:].rearrange("b (nb nl) h -> (b nb) nl h", nl=64), cl[:])

    _split_multi_waits(nc)
    return nc


def _host_prep(features, W1, b1, Wb, bb, Wc, bc, ucb_count, counter):
    features = np.ascontiguousarray(features, dtype=np.float32)
    W1 = np.asarray(W1, dtype=np.float32)
    b1 = np.asarray(b1, dtype=np.float32)
    Wb = np.asarray(Wb, dtype=np.float32)
    bb = np.asarray(bb, dtype=np.float32)
    Wc = np.asarray(Wc, dtype=np.float32)
    bc = np.asarray(bc, dtype=np.float32)
    ucb_count = np.asarray(ucb_count, dtype=np.float32)

    W1mat = W1.transpose(2, 0, 1).reshape(F, COLS)           # [f, col], col=h*HD+d
    W1m8 = W1mat.reshape(8, 128, COLS)
    import ml_dtypes
    W1mb = np.ascontiguousarray(W1m8).astype(ml_dtypes.bfloat16)
    # rescue layout: [h, p, k, d] = W1mat[k*128+p, h*256+d]
    W1m = np.ascontiguousarray(W1m8.reshape(8, 128, 8, 256).transpose(2, 1, 0, 3))
    b1rep = np.ascontiguousarray(np.tile(b1.reshape(1, COLS), (16, 1)))
    Wbrep_row = np.zeros((COLS,), np.float32)
    for h in range(H):
        Wbrep_row[h * HD:(h + 1) * HD] = Wb[h, 0, :]
    Wbrep = np.ascontiguousarray(np.tile(Wbrep_row.reshape(1, COLS), (16, 1)))
    b1flat = b1.reshape(COLS)
    b1T = np.ascontiguousarray(b1flat.reshape(16, 128).T)
    Wbig = np.zeros((COLS, 8), np.float32)
    for h in range(H):
        Wbig[h * HD:(h + 1) * HD, h] = Wb[h, 0, :]
    Wbg = np.ascontiguousarray(Wbig.reshape(16, 128, 8))
    Wbm = np.ascontiguousarray(Wb[:, 0, :].reshape(8, 2, 128).transpose(2, 1, 0))
    bbv = np.ascontiguousarray(bb.reshape(8, 1))
    bb16 = np.ascontiguousarray(np.tile(bb.reshape(1, 8), (2, 1)).reshape(16, 1))

    # ucb constant per (b,h): replicate reference fp32 ops
    Ct = ucb_count.transpose(0, 3, 2, 1)                     # (B,H,R,N)
    ssum = Ct.sum(axis=-1, dtype=np.float32) + np.float32(1e-6)   # (B,H,R)
    log_iter = np.float32(np.log(max(1, int(counter))))
    cub = np.sqrt((log_iter / ssum).astype(np.float32)).astype(np.float32)  # BETA=1
    cvec = np.ascontiguousarray(cub.reshape(16, 1))

    eqpA = np.zeros((128, 16), np.float32)
    eqpB = np.zeros((128, 16), np.float32)
    for p in range(128):
        eqpA[p, p // 16] = 1.0
        if (p + 128) // 16 < 16:
            eqpB[p, (p + 128) // 16] = 1.0

    shared = dict(W1m=W1m, W1mb=W1mb, b1rep=b1rep, Wbrep=Wbrep, b1T=b1T,
                  Wbg=Wbg, Wbm=Wbm, bbv=bbv, bb16=bb16,
                  cvec=cvec, eqpA=eqpA, eqpB=eqpB)

    in_maps = []
    for c in range(NCORES):
        fshard = np.ascontiguousarray(features[:, c * NL:(c + 1) * NL, :])
        cshard = np.ascontiguousarray(ucb_count[:, c * NL:(c + 1) * NL, 0, :])
        Wcs = Wc[c * 128:(c + 1) * 128, :]                   # [128 fo, 8192]
        WcT = np.ascontiguousarray(
            Wcs.T.reshape(8, 8, 128, 128).transpose(0, 2, 1, 3).reshape(8, 128, 1024))
        bcs = np.ascontiguousarray(bc[c * 128:(c + 1) * 128].reshape(128, 1))
        bcrep = np.ascontiguousarray(np.tile(bc[c * 128:(c + 1) * 128].reshape(1, 128), (2, 1)))
        meta = np.zeros((128, 4), np.float32)
        for p in range(128):
            meta[p, 0] = c * NL + (p % 64) * 64              # nat-layout row base
            meta[p, 1] = c * NL                              # coreoff
            meta[p, 3] = c * NL + NL
        for p in range(16):
            meta[p, 2] = (p // 8) * NL                       # b*NL for [16,x] rows
        m = dict(shared)
        m.update(feat=fshard, cnt=cshard, WcT=WcT, bcs=bcs, bcrep=bcrep, meta=meta)
        in_maps.append(m)
    return in_maps


def kernel(features, W1, b1, Wb, bb, Wc, bc, ucb_count, counter):
    if "nc" not in _CACHE:
        _CACHE["nc"] = _build()
    nc = _CACHE["nc"]
    in_maps = _host_prep(features, W1, b1, Wb, bb, Wc, bc, ucb_count, counter)
    res = run_bass_kernel_spmd(nc, in_maps, list(range(NCORES)))
    rs = res.results

    head_attentions = np.empty((B, 1, H, N), np.float32)
    for c in range(NCORES):
        s = rs[c]["S_out"]                                   # [16, NL]
        head_attentions[:, 0, :, c * NL:(c + 1) * NL] = s.reshape(2, 8, NL)
    new_count = np.empty((B, N, 1, H), np.float32)
    for c in range(NCORES):
        new_count[:, c * NL:(c + 1) * NL, 0, :] = rs[c]["cnt_out"]
    agg = np.empty((B, 1, F), np.float32)
    for c in range(NCORES):
        agg[:, 0, c * 128:(c + 1) * 128] = rs[c]["agg_out"]  # [2, 128 fo]
    return agg, head_attentions, new_count
